# revision 19
# baseline (speedup 1.0000x reference)
"""Trainium2 Bass kernel: 7x7 valid 2D cross-correlation on a 6144x6144 fp32
image, + scalar bias. Output 6138x6138 fp32.

Strategy (v2: column-interleaved packing, bf16 I/O)
---------------------------------------------------
Row-band sharding across 8 NeuronCores: core c computes output rows
[c*768, c*768+768) (core 7's bottom 6 rows dropped on gather).

The host pre-interleaves the input band into XI[6*R + cs, t] = x[R, 6*t + cs]
(bf16), so one SBUF tile [126, 1024] holds a 21-image-row x 6144-column block
with partition p = 6*r + cs carrying (row r, column-phase cs). One stationary
matrix then encodes all 7 row taps AND 6 of the column-phase combinations at
once:

    U_s[6*(mr+d) + (mc+j)%6, 6*mr+mc] = W[d, j]   for s = (mc+j)//6 in {0,1}

so a block of 15 output rows x 6138 columns needs only TWO full-width matmul
streams (free-dim shift s = 0, 1) instead of 7 -- 2046 PE cycles per block,
106k cycles/core (vs 300k for the row-band scheme). PSUM accumulates the two
streams in fp32. 6144 = 6*1024 and 6138 = 6*1023, so tiles have zero padding.

I/O is bf16 both ways (tolerance is 2e-2; measured error ~2e-3): input
9.5 MB (+40% block halo re-read = 13.2 MB) + output 9.4 MB per core ~= 63 us
at the ~358 GB/s per-core HBM limit, which is the design bottleneck. Output
rows are stored interleaved (YI[6*row + mc, n] = y[row, 6*n + mc]) and
de-interleaved + upcast + bias-added on the host.

Eviction PSUM->SBUF (with fp32->bf16 convert) is split DVE (cols 0:512) /
ACT (cols 512:1023) so neither engine becomes the bottleneck. Loads alternate
the two HWDGE rings (sync/scalar); stores go through SWDGE (gpsimd) split
into sub-DMAs so several SDMA engine pairs drain them concurrently.
"""

import os

import numpy as np
import ml_dtypes

import concourse.tile as tile
from concourse import bacc, mybir
from concourse.bass_utils import run_bass_kernel_spmd

BF16 = ml_dtypes.bfloat16

H = 6144
W = 6144
KH = 7
KW = 7
OH = H - KH + 1          # 6138
OW = W - KW + 1          # 6138
NCORES = 8
RPC = 768                # output rows per core (8*768 = 6144; last 6 dropped)
IRPC = RPC + KH - 1      # 774 input rows per core
CP = 6                   # column phases packed into partitions
MR = 15                  # output rows per block
KR = MR + KH - 1         # 21 input rows per block
KP = CP * KR             # 126 contraction partitions
MQ = CP * MR             # 90 live output partitions (stationary padded to 128)
NT = W // CP             # 1024 free-dim columns per input tile
NO = OW // CP            # 1023 output columns per phase
NBLK = 52                # 51 full blocks + 1 partial (offset 753, store 3 rows)
LAST_RB = RPC - MR       # 753

_NC_CACHE = {}
LAST_RESULTS = None      # for the local test harness; the grader ignores this


def _build_nc():
    f32 = mybir.dt.float32
    bf16 = mybir.dt.bfloat16

    nc = bacc.Bacc(trn_type="TRN2", target_bir_lowering=False, debug=False,
                   num_devices=NCORES)
    xi = nc.dram_tensor("xi", [CP * IRPC, NT], bf16, kind="ExternalInput")
    bands = nc.dram_tensor("bands", [KP, 256], bf16, kind="ExternalInput")
    y = nc.dram_tensor("y", [CP * RPC, NO], bf16, kind="ExternalOutput")

    with tile.TileContext(nc) as tc:
        with tc.tile_pool(name="const", bufs=1) as constp, \
             tc.tile_pool(name="xin", bufs=8) as xp, \
             tc.tile_pool(name="psum", bufs=3, space="PSUM") as pp, \
             tc.tile_pool(name="scrap", bufs=1, space="PSUM") as scrapp, \
             tc.tile_pool(name="outs", bufs=12) as op:
            ub = constp.tile([KP, 256], bf16)
            # Warm-up stationary that does NOT depend on any DMA (memset on
            # the idle DVE) so the PE HAM clock-gate ramp starts ~3 us before
            # block 0's data lands, instead of waiting out the ub load's
            # completion receipt.
            wtile = constp.tile([KP, 256], bf16)
            nc.vector.memset(wtile[:], 1.0)

            # Prologue loads: block 0 split across both HWDGE rings so its
            # first matmuls start early; block 1 on sync behind it.
            xins = {}
            x0 = constp.tile([KP, NT], bf16)
            xins[0] = x0
            nc.sync.dma_start(xins[0][0:63, :], xi[0:63, :])
            nc.scalar.dma_start(xins[0][63:KP, :], xi[63:KP, :])
            # ub loads via the otherwise-idle SWDGE path, in parallel with
            # the x0 halves on the two HWDGE rings.
            nc.gpsimd.dma_start(ub[:], bands[:])

            def load_block(bl):
                # Halo-free load: only the 90 new partition-rows come from
                # HBM; the 36-row halo is copied SBUF->SBUF from the tail of
                # the previous block's tile (scalar ring), cutting HBM input
                # traffic by 29%.
                rbl = MR * bl if bl < NBLK - 1 else LAST_RB
                xnext = xp.tile([KP, NT], bf16, tag="xin")
                xins[bl] = xnext
                nc.sync.dma_start(xnext[36:KP, :],
                                  xi[CP * rbl + 36:CP * rbl + KP, :])
                p0 = 90 if bl < NBLK - 1 else 18
                nc.scalar.dma_start(xnext[0:36, :],
                                    xins[bl - 1][p0:p0 + 36, :])
            # Warm-up burst on the stationary tile while block 0 loads, so
            # the PE HAM clock-gate reaches 8/8 before the real stream.
            # Targets block 0's own PSUM tile (overwritten by its start=True
            # matmul) so no extra PSUM bank is consumed. Emitted before the
            # prologue loads so the ub load's completion lane isn't aliased
            # behind them.
            ps0A = pp.tile([128, 512], f32, tag="psA")
            ps0B = pp.tile([128, 511], f32, tag="psB")
            scrap = scrapp.tile([128, 512], f32)
            for i in range(8):
                nc.tensor.matmul(scrap[:, 0:256], wtile[:, 0:128],
                                 wtile[:, 0:256], start=(i == 0),
                                 stop=(i == 7))

            for pb in (1, 2, 3):
                load_block(pb)

            for b in range(NBLK):
                rb = MR * b if b < NBLK - 1 else LAST_RB
                # Four-block load lookahead keeps xin prefetch well ahead
                # of the consuming matmuls.
                bl = b + 4
                if bl < NBLK:
                    load_block(bl)
                xin = xins[b]
                xins.pop(b - 1, None)
                if b == 0:
                    psA, psB = ps0A, ps0B
                else:
                    psA = pp.tile([128, 512], f32, tag="psA")
                    psB = pp.tile([128, 511], f32, tag="psB")
                # Interleave the two PSUM tiles per stream so only two
                # weight switches happen per block (LDWEIGHTS hides behind
                # the previous matmul via the PE reorder window).
                nc.tensor.matmul(psA[:], ub[:, 0:128], xin[:, 0:512],
                                 start=True, stop=False)
                nc.tensor.matmul(psA[:], ub[:, 128:256], xin[:, 1:513],
                                 start=False, stop=True)
                nc.tensor.matmul(psB[:], ub[:, 0:128], xin[:, 512:1023],
                                 start=True, stop=False)
                nc.tensor.matmul(psB[:], ub[:, 128:256], xin[:, 513:1024],
                                 start=False, stop=True)
                # Two pacer matmuls into the scrap bank stretch the PE's
                # per-block time to ~1.3 us, matching the HBM-feasible rate.
                # A PE-paced pipeline never sprints ahead of the store
                # stream, so it never hits multi-us buffer-full stalls, and
                # the HAM clock gate stays at 8/8 for the whole run (a cold
                # block costs 2x). Net win despite the wasted cycles.
                for w in range(2):
                    nc.tensor.matmul(scrap[:], wtile[:, 0:128],
                                     xin[:, 0:512], start=True, stop=True)
                ot = op.tile([MQ, NO], bf16)
                nc.vector.tensor_scalar_add(ot[:, 0:512], psA[0:MQ, :], 0.0)
                nc.scalar.copy(ot[:, 512:NO], psB[0:MQ, :])
                # One SWDGE store per block: async trigger (~0.7 us of Q7
                # time), transfers drain at ~150 GB/s without blocking any
                # compute queue.
                if b < NBLK - 1:
                    if b < NBLK - 6:
                        nc.gpsimd.dma_start(y[CP * rb:CP * rb + MQ, :],
                                            ot[:])
                    else:
                        # Loads are done by now; the idle HWDGE rings drain
                        # the final stores in parallel with the SWDGE
                        # backlog, shortening the end-of-kernel DRAIN.
                        eng = nc.sync if b % 2 == 0 else nc.scalar
                        eng.dma_start(y[CP * rb:CP * rb + MQ, :], ot[:])
                else:
                    # Only output rows 765..767 (q in [72, 90)) are new.
                    nc.gpsimd.dma_start(y[CP * rb + 72:CP * rb + MQ, :],
                                        ot[72:MQ, :])
    nc.compile()
    return nc


def _get_nc():
    if "v2" not in _NC_CACHE:
        _NC_CACHE["v2"] = _build_nc()
    return _NC_CACHE["v2"]


def _build_bands(weight: np.ndarray) -> np.ndarray:
    """U_s[6*(mr+d) + (mc+j)%6, 128*s + 6*mr+mc] = weight[d, j]."""
    U = np.zeros((KP, 256), dtype=np.float32)
    for mr in range(MR):
        for mc in range(CP):
            q = CP * mr + mc
            for d in range(KH):
                for j in range(KW):
                    s, cs = divmod(mc + j, CP)
                    U[CP * (mr + d) + cs, 128 * s + q] = weight[d, j]
    return U.astype(BF16)


def kernel(x: np.ndarray, weight: np.ndarray, bias: np.ndarray) -> np.ndarray:
    global LAST_RESULTS
    trace = os.environ.get("CONV_TRACE", "") == "1"

    xs = np.asarray(x, dtype=np.float32)
    assert xs.shape == (H, W), xs.shape
    bands = _build_bands(np.asarray(weight, dtype=np.float32))
    bval = float(np.asarray(bias, dtype=np.float32).reshape(-1)[0])

    xpad = np.zeros((NCORES * RPC + KH - 1, W), dtype=np.float32)
    xpad[:H, :] = xs
    in_maps = []
    for c in range(NCORES):
        band = xpad[c * RPC:c * RPC + IRPC, :].astype(BF16)
        xi = band.reshape(IRPC, NT, CP).transpose(0, 2, 1).reshape(
            CP * IRPC, NT)
        in_maps.append({"xi": np.ascontiguousarray(xi), "bands": bands})

    nc = _get_nc()
    kwargs = {}
    if trace:
        kwargs = dict(trace=True, trace_cores=[0])
    res = run_bass_kernel_spmd(nc, in_maps, core_ids=list(range(NCORES)),
                               **kwargs)
    LAST_RESULTS = res
    rows = []
    for r in res.results:
        yi = np.asarray(r["y"])                       # [4608, 1023] bf16
        yc = yi.reshape(RPC, CP, NO).transpose(0, 2, 1).reshape(RPC, OW)
        rows.append(yc)
    out = np.concatenate(rows, axis=0)[:OH].astype(np.float32)
    if bval != 0.0:
        out += bval
    return np.ascontiguousarray(out)
